# revision 58
# baseline (speedup 1.0000x reference)
"""Trainium2 Bass kernel for nn_Attention_17532056502607.

Multi-head self-attention (B=8, N=48*48=2304 tokens, C=384, 8 heads of 48):
    q = x @ q_w.T + q_b ; k,v = x @ kv_w.T + kv_b
    out = softmax(q k^T / sqrt(48)) v ; y = out @ proj_w.T + proj_b

Sharding: data-parallel, one batch element per NeuronCore (8 cores).

The kernel is ACT(exp)-throughput bound: 8 heads x 2304^2 scores = 42.5M
exps at 1 elem/cyc/lane @1.2GHz is a ~280us floor on the Scalar engine.
The design keeps one continuous exp stream on ACT and hides ALL other
work (projections, attn@V, output proj, normalize) under it:

  - S^T layout: scores [keys, q] via kT-tile.T @ qT, head pairs packed at
    partition rows 0-47 / 64-111 so row-group-disjoint matmuls stream
    concurrently in the PE array.
  - exp groups of 3x512 (or 6x256) PSUM banks, double-buffered (2 tags),
    so S^T(g+2) emission only waits exp(g): ACT never starves.
  - attn@V accumulates into a SINGLE psum bank: head a at rows 0-48,
    head b at rows 64-112 (col-group-disjoint -> concurrent), with a
    ones-column per head producing the softmax denominator at rows 0/64.
  - the attn@V queue is delayed 3 groups and spans chunk/pair boundaries
    so the exp stream never flushes.
  - x->q/k/v projections (bf16, FWL) and the output projection run as
    "filler" PE work popped one unit per exp group; v and pair-0 q/k run
    up front, pair p+1's q/k during pair p, proj during pair 3.
  - normalize: 1 DVE copy of rows 0:113, reciprocal_approx_fast of the
    two denominator rows, one rank-2 selector matmul broadcasts both
    reciprocals, one DVE multiply into bf16 oT_sb.
"""

import os
import sys

import numpy as np

for _p in ("/opt/trn_rl_repo",):
    if _p not in sys.path:
        sys.path.append(_p)

import concourse.bass as bass  # noqa: E402
import concourse.tile as tile  # noqa: E402
from concourse import bacc, mybir  # noqa: E402
from concourse.bass_utils import run_bass_kernel_spmd  # noqa: E402

# ---------------------------------------------------------------- constants
B = 8
HH = 48
WW = 48
C = 384
N = HH * WW  # 2304
NH = 8
HD = 48
PAIRS = NH // 2  # 4
P = 128
NT = N // P  # 18 token tiles
KTC = C // P  # 3 contraction tiles over C
SCALE = float(HD) ** -0.5
VW = NH * (HD + 1)  # 392: v with a ones column per head
CHUNKS = [(0, 512), (512, 512), (1024, 512), (1536, 512), (2048, 256)]

F32 = mybir.dt.float32
BF16 = mybir.dt.bfloat16
F32R = mybir.dt.float32r

# Input-side matmul dtype (x, q/k/v weights): bf16 halves DMA/SBUF and
# enables FWL weight loads.
XT_DT = getattr(mybir.dt, os.environ.get("ATTN_XT_DT", "bfloat16"))
# S^T operands (qT/kT) and attn@V operands (v, est).
ST_DT = getattr(mybir.dt, os.environ.get("ATTN_ST_DT", "bfloat16"))
AV_DT = getattr(mybir.dt, os.environ.get("ATTN_AV_DT", "bfloat16"))
# Normalized attention output + proj weights.
OT_DT = getattr(mybir.dt, os.environ.get("ATTN_OT_DT", "bfloat16"))
# Broadcast-matmul operands (rank-1 selectors): f32r streams at 1 cyc/col
# on the PE (fp32 is 4). The reciprocal writes f32r directly under
# allow_low_precision so the verifier sees a rounded producer.
BC_DT = F32R

_EXP = mybir.ActivationFunctionType.Exp


def _groups_for(parity):
    """List of (gs, tag) exp groups covering the 36 S^T tiles of a chunk.
    Tag 'A' tiles span 3 PSUM banks, tag 'B' tiles span 2 (5 banks total,
    strictly alternating — including across chunk boundaries via the
    parity flip — so consecutive groups never wait on each other's exp).
    Tiles are always 512 wide — at qw=256 each matmul still gets a full
    bank to itself (two concurrent row-group matmuls must not share a
    PSUM bank)."""
    if parity == 0:
        return [(3, "A"), (2, "B")] * 7 + [(1, "A")]
    return [(2, "B"), (3, "A")] * 7 + [(1, "B")]


def _emit(tc: tile.TileContext, d: dict, ctx):
    nc = tc.nc

    persist = ctx.enter_context(tc.tile_pool(name="persist", bufs=1))
    v_sb = persist.tile([P, NT, VW], AV_DT, name="v_sb")
    qT_sb = persist.tile([P, PAIRS, N], ST_DT, name="qT_sb")
    kT_sb = persist.tile([P, PAIRS, N], ST_DT, name="kT_sb")
    oT_sb = persist.tile([P, PAIRS, N], OT_DT, name="oT_sb")
    pw_sb = persist.tile([P, PAIRS, C], OT_DT, name="pw_sb")
    qb_sb = persist.tile([P, PAIRS], F32, name="qb_sb")
    kb_sb = persist.tile([P, PAIRS], F32, name="kb_sb")
    vbB_sb = persist.tile([P, VW], F32, name="vbB_sb")
    pbB_sb = persist.tile([P, C], F32, name="pbB_sb")
    sel2_sb = persist.tile([1, 256], BC_DT, name="sel2_sb")
    xT_sb = persist.tile([P, KTC, N], XT_DT, name="xT_sb")
    wq_sb = persist.tile([P, KTC, PAIRS * P], XT_DT, name="wq_sb")
    wk_sb = persist.tile([P, KTC, PAIRS * P], XT_DT, name="wk_sb")
    wv_sb = persist.tile([P, KTC, VW], XT_DT, name="wv_sb")

    # DMAs ordered by first use: v batch 0 needs xT slice kt and wv.
    for kt in range(KTC):
        nc.sync.dma_start(
            xT_sb[:, kt, :], d["xT"][kt * P : (kt + 1) * P, :]
        )
    nc.sync.dma_start(wv_sb[:], d["wvA"].rearrange("(kt p) m -> p kt m", p=P))
    nc.sync.dma_start(vbB_sb[:], d["vbB"])
    nc.sync.dma_start(wq_sb[:], d["wqP"].rearrange("(kt p) m -> p kt m", p=P))
    nc.sync.dma_start(wk_sb[:], d["wkP"].rearrange("(kt p) m -> p kt m", p=P))
    nc.sync.dma_start(qb_sb[:], d["qbP"])
    nc.sync.dma_start(kb_sb[:], d["kbP"])
    # (sel2/pw/pbB DMAs are emitted after phase A: they are needed much
    # later, and phase-A matmuls wait on the shared DMA semaphore.)

    # oT_sb rows 49-63 are never written by the normalize muls: zero the
    # whole tensor once, at t=0 while the DVE is idle (the proj matmul
    # contracts 0-weights against these rows). Also fire a tiny dummy exp
    # so the ~2.7us ACT table load happens during phase A, not at the
    # first real softmax.
    nc.vector.memset(oT_sb[:], 0.0)
    warm = persist.tile([1, 8], F32, name="warm")
    nc.vector.memset(warm[:], 0.0)
    nc.scalar.activation(warm[:], warm[:], _EXP, scale=1.0)

    # ------------- helpers emitting one "unit" of projection work ---------
    def qk_unit(pool, tag, wt, bias, dst, pr, q0, qw):
        ps = pool.tile([P, 512], F32, name="psqk", tag=tag)
        for kt in range(KTC):
            nc.tensor.matmul(
                ps[:, 0:qw],
                lhsT=wt[:, kt, pr * P : (pr + 1) * P],
                rhs=xT_sb[:, kt, q0 : q0 + qw],
                start=(kt == 0),
                stop=(kt == KTC - 1),
            )
        nc.vector.tensor_scalar_add(
            dst[:, pr, q0 : q0 + qw], ps[:, 0:qw], bias[:, pr : pr + 1]
        )

    def v_unit(pool, tag, nt):
        psv = pool.tile([P, 512], F32, name="psv", tag=tag)
        for kt in range(KTC):
            nc.tensor.matmul(
                psv[:, 0:VW],
                lhsT=xT_sb[:, kt, nt * P : (nt + 1) * P],
                rhs=wv_sb[:, kt, :],
                start=(kt == 0),
                stop=(kt == KTC - 1),
            )
        nc.vector.tensor_add(v_sb[:, nt, :], psv[:, 0:VW], vbB_sb[:])

    # ---------------- phase A upfront: v batch 0, kT pair 0, qT c0/c1 -----
    DBG_NOSTREAM = os.environ.get("ATTN_DBG_NOSTREAM") == "1"
    with tc.tile_pool(name="phA", bufs=1, space="PSUM") as psA:
        # v token tiles 0-5, kt-outer across 6 open accumulators so the
        # first matmuls start as soon as xT slice 0 lands.
        nbat = 3 if DBG_NOSTREAM else 1
        for b in range(nbat):
            psv = [
                psA.tile([P, 512], F32, name=f"psv{i}", tag=f"b{i}")
                for i in range(6)
            ]
            for kt in range(KTC):
                for i in range(6):
                    nc.tensor.matmul(
                        psv[i][:, 0:VW],
                        lhsT=xT_sb[:, kt, (b * 6 + i) * P : (b * 6 + i + 1) * P],
                        rhs=wv_sb[:, kt, :],
                        start=(kt == 0),
                        stop=(kt == KTC - 1),
                    )
            for i in range(6):
                nc.vector.tensor_add(
                    v_sb[:, b * 6 + i, :], psv[i][:, 0:VW], vbB_sb[:]
                )
        # kT pair 0 chunks 0-1 and qT chunk 0 only: chunk-0's S^T reaches
        # key-chunk c2 at group ~5, so kT c2-c4 (and qT c1) ride the early
        # filler slots instead of delaying the first exp.
        for q0, qw in CHUNKS[:2]:
            qk_unit(psA, "u0", wk_sb, kb_sb, kT_sb, 0, q0, qw)
        for q0, qw in CHUNKS[:1]:
            qk_unit(psA, "u1", wq_sb, qb_sb, qT_sb, 0, q0, qw)
        if DBG_NOSTREAM:
            for q0, qw in CHUNKS[2:]:
                qk_unit(psA, "u1", wq_sb, qb_sb, qT_sb, 0, q0, qw)
            for prn in range(1, PAIRS):
                for q0, qw in CHUNKS:
                    qk_unit(psA, "u0", wk_sb, kb_sb, kT_sb, prn, q0, qw)
                    qk_unit(psA, "u1", wq_sb, qb_sb, qT_sb, prn, q0, qw)

    # late DMAs: first consumed by the normalize (~40us) and proj (~100us)
    nc.sync.dma_start(sel2_sb[:], d["selE2"])
    nc.sync.dma_start(pw_sb[:], d["pwP"].rearrange("r p m -> p r m"))
    nc.sync.dma_start(pbB_sb[:], d["pbR"])

    # ---------------- fused attention + filler stream ---------------------
    with (
        tc.tile_pool(name="es", bufs=5) as es_pool,
        tc.tile_pool(name="rc", bufs=4) as rc_pool,
        tc.tile_pool(name="fin", bufs=3) as fin_pool,
        tc.tile_pool(name="psS", bufs=1, space="PSUM") as psS,
        tc.tile_pool(name="psO", bufs=1, space="PSUM") as psO,
        tc.tile_pool(name="psX", bufs=1, space="PSUM") as psX,
    ):
        # attn@V accumulator banks: head a at rows 0-48 of oTA, head b at
        # rows 64-112 of oTB (disjoint col groups -> concurrent matmuls;
        # separate banks because a PSUM bank allows one open accumulation
        # group at a time). oT_sb rows 49-63 are never written: zero once
        # so the proj matmul contracts 0-weights against 0, not garbage.
        oTA = psO.tile([P, 512], F32, name="oTA", tag="oTA")
        oTB = psO.tile([P, 512], F32, name="oTB", tag="oTB")
        # seed rows 49-63 of both oU slots with 1.0 once: the drain copies
        # never write them, and the [0:65] batched reciprocal must not see
        # zeros (1/0 faults the DVE) — the baseline seeded its den tiles
        # the same way.
        for _i in range(4):
            _oUi = rc_pool.tile([P, 512], F32, name=f"oUinit{_i}", tag="oU")
            nc.vector.memset(_oUi[32:64, :], 1.0)

        def fF_unit(nt):
            fF = psX.tile([P, C], F32, name="fF", tag="scr")
            for pr2 in range(PAIRS):
                nc.tensor.matmul(
                    fF[:],
                    lhsT=oT_sb[0:113, pr2, nt * P : (nt + 1) * P],
                    rhs=pw_sb[0:113, pr2, :],
                    start=(pr2 == 0),
                    stop=(pr2 == PAIRS - 1),
                )
            ft = fin_pool.tile([P, C], F32, name="ft", tag="ft")
            nc.vector.tensor_add(ft[:], fF[:], pbB_sb[:])
            nc.sync.dma_start(d["out"][nt * P : (nt + 1) * P, :], ft[:])

        # filler queue: each entry emits one small batch of PE work whose
        # inputs are already (or will shortly be) available. Consumers of
        # a filler's output must be EMITTED after it (in-order PE queue).
        fillers = []
        if not DBG_NOSTREAM:
            # kT c2-c4 + qT c1 first (consumed by chunk-0 groups >=5 and
            # chunk 1), then v tiles 6-17 (chunk-0's attn@V sweep), then
            # qT chunks 2-4.
            for q0, qw in CHUNKS[2:]:
                fillers.append(
                    lambda q0=q0, qw=qw: qk_unit(
                        psX, "scr", wk_sb, kb_sb, kT_sb, 0, q0, qw
                    )
                )
            q1, w1 = CHUNKS[1]
            fillers.append(
                lambda: qk_unit(psX, "scr", wq_sb, qb_sb, qT_sb, 0, q1, w1)
            )
            for nt in range(6, NT):
                fillers.append(lambda nt=nt: v_unit(psX, "scr", nt))
            for q0, qw in CHUNKS[2:]:
                fillers.append(
                    lambda q0=q0, qw=qw: qk_unit(
                        psX, "scr", wq_sb, qb_sb, qT_sb, 0, q0, qw
                    )
                )
        # pair p+1's k/q during pair p, proj during pair 3: appended below.

        seq = [(kt, hoff) for kt in range(NT) for hoff in (0, 64)]
        av_q = []  # delayed attn@V emissions, spans chunk/pair boundaries
        pend = []  # queued normalize closures (emitted 2 chunks later)
        recq = []  # queued reciprocal closures (emitted 1 chunk later)

        def attnv(est, si, gs, pr, qw, after):
            for j in range(gs):
                kt2, hoff2 = seq[si + j]
                h = pr * 2 + (0 if hoff2 == 0 else 1)
                oT = oTA if hoff2 == 0 else oTB
                nc.tensor.matmul(
                    oT[hoff2 : hoff2 + HD + 1, 0:qw],
                    lhsT=v_sb[:, kt2, h * (HD + 1) : (h + 1) * (HD + 1)],
                    rhs=est[:, j, 0:qw],
                    start=(kt2 == 0),
                    stop=(kt2 == NT - 1),
                )
            if after is not None:
                after()

        def make_drain(pr, q0, qw):
            # emitted right after the chunk's last attn@V: DVE copies of the
            # two head blocks, then fast approximate reciprocals of the two
            # denominator rows. The normalize (PE broadcast + DVE muls) is
            # deferred via pend[].
            def drain():
                # only the two copies sit on the oTA/oTB WAR path (the
                # next chunk's attn@V waits on them); the reciprocal reads
                # the SBUF copy so the accumulator banks free immediately,
                # and is emitted at the END of the next chunk's group loop
                # so no filler add (whose completion gates the scr-bank
                # WAR for the bc matmul) ever queues behind its ~3.4us
                # DVE occupancy. (reciprocal_approx_fast is NOT used: the
                # custom-DVE op executes as garbage through this runtime.)
                oU = rc_pool.tile([P, 512], F32, name="oU", tag="oU")
                nc.vector.tensor_copy(oU[0 : HD + 1, 0:qw], oTA[0 : HD + 1, 0:qw])
                nc.vector.tensor_copy(
                    oU[64 : 64 + HD + 1, 0:qw], oTB[64 : 64 + HD + 1, 0:qw]
                )
                rec = rc_pool.tile([P, 512], BC_DT, name="rec", tag="rec")
                rec1 = rc_pool.tile([1, 512], BC_DT, name="rec1", tag="rec1")
                # one reciprocal spans both denominator rows (0, 64);
                # rows 1-63 are junk reciprocals, never read. ([1,512]
                # reciprocals measure ~2x slower than this shape.)
                with nc.allow_low_precision(
                    reason="float32r keeps fp32 bits; PE rounds on read"
                ):
                    nc.vector.reciprocal(rec[0:65, 0:qw], oU[0:65, 0:qw])
                # recB moved to a base-0 row: both selector matmuls then
                # keep lhsT/rhs/dst at partition base 0 (f32r cannot
                # write PSUM at base 64).
                nc.vector.tensor_copy(rec1[0:1, 0:qw], rec[64:65, 0:qw])

                def normalize():
                    if os.environ.get("ATTN_DBG_NONORM") == "1":
                        nc.vector.tensor_copy(
                            oT_sb[0 : HD + 1, pr, q0 : q0 + qw], oU[0 : HD + 1, 0:qw]
                        )
                        nc.vector.tensor_copy(
                            oT_sb[64 : 64 + HD + 1, pr, q0 : q0 + qw],
                            oU[64 : 64 + HD + 1, 0:qw],
                        )
                        return
                    # rank-1 selector matmuls broadcast recA to bc rows
                    # 0-48 and recB to rows 64-112, accumulating into a
                    # base-0 dst (f32r cannot write PSUM at base 64).
                    bc = psX.tile([P, 512], F32, name="bc", tag="scr")
                    nc.tensor.matmul(
                        bc[0:113, 0:qw],
                        lhsT=sel2_sb[0:1, 0:113],
                        rhs=rec[0:1, 0:qw],
                        start=True,
                        stop=False,
                    )
                    nc.tensor.matmul(
                        bc[0:113, 0:qw],
                        lhsT=sel2_sb[0:1, 128:241],
                        rhs=rec1[0:1, 0:qw],
                        start=False,
                        stop=True,
                    )
                    nc.vector.tensor_mul(
                        oT_sb[0 : HD + 1, pr, q0 : q0 + qw],
                        oU[0 : HD + 1, 0:qw],
                        bc[0 : HD + 1, 0:qw],
                    )
                    nc.vector.tensor_mul(
                        oT_sb[64 : 64 + HD + 1, pr, q0 : q0 + qw],
                        oU[64 : 64 + HD + 1, 0:qw],
                        bc[64 : 64 + HD + 1, 0:qw],
                    )

                pend.append(normalize)

            return drain

        _maxit = int(os.environ.get("ATTN_DBG_MAXIT", "20"))
        _chunks = list(CHUNKS)
        if os.environ.get("ATTN_DBG_DUP512") == "1":
            _chunks[-1] = _chunks[-2]
        _it = 0
        for pr in range(PAIRS):
            for ci, (q0, qw) in enumerate(_chunks):
                if _it >= _maxit:
                    break
                _it += 1
                groups = _groups_for((pr * len(CHUNKS) + ci) % 2)
                pend_idx = len(groups) - 3
                si = 0
                for g, (gs, btag) in enumerate(groups):
                    if g == pend_idx and len(pend) >= 2:
                        pend.pop(0)()
                        if pr == PAIRS - 1 and ci >= 2:
                            # proj for the chunk just normalized
                            pq0, pqw = CHUNKS[ci - 2]
                            for nt in range(pq0 // P, (pq0 + pqw) // P):
                                fillers.append(lambda nt=nt: fF_unit(nt))
                    sg = psS.tile(
                        [P, gs, 512], F32, name="sg", tag=f"sg{btag}"
                    )
                    for j in range(gs):
                        kt, hoff = seq[si + j]
                        nc.tensor.matmul(
                            sg[:, j, 0:qw],
                            lhsT=kT_sb[hoff : hoff + HD, pr, kt * P : (kt + 1) * P],
                            rhs=qT_sb[hoff : hoff + HD, pr, q0 : q0 + qw],
                            start=True,
                            stop=True,
                        )
                    est = es_pool.tile([P, gs, 512], AV_DT, name="est", tag="est")
                    nc.scalar.activation(
                        est[:, :, 0:qw], sg[:, :, 0:qw], _EXP, scale=SCALE
                    )
                    after = (
                        make_drain(pr, q0, qw) if g == len(groups) - 1 else None
                    )
                    av_q.append((est, si, gs, pr, qw, after))
                    si += gs
                    if len(av_q) > 3:
                        attnv(*av_q.pop(0))
                    # keep boundary slots clean: the PE must race ahead on
                    # the next chunk's first S^T groups and the queued
                    # attn@V tail there.
                    if g >= 2 and fillers:
                        fillers.pop(0)()
                        # first chunk: double-pop early slots so the v
                        # tiles keep pace with the attn@V sweep despite
                        # the 4 prepended kT/qT units
                        if pr == 0 and ci == 0 and g <= 5 and fillers:
                            fillers.pop(0)()
                if DBG_NOSTREAM:
                    # flush the attn@V queue at every chunk boundary
                    for av in av_q:
                        attnv(*av)
                    av_q = []
                # queue pair pr+1's projections during pair pr's chunks
                # (2 units per chunk: all 10 done before pair pr+1 starts)
                if not DBG_NOSTREAM and pr < PAIRS - 1:
                    q0n, qwn = CHUNKS[ci]
                    fillers.append(
                        lambda q0n=q0n, qwn=qwn, prn=pr + 1: qk_unit(
                            psX, "scr", wk_sb, kb_sb, kT_sb, prn, q0n, qwn
                        )
                    )
                    fillers.append(
                        lambda q0n=q0n, qwn=qwn, prn=pr + 1: qk_unit(
                            psX, "scr", wq_sb, qb_sb, qT_sb, prn, q0n, qwn
                        )
                    )

        # tail: flush the attn@V queue, the two queued normalizes, and the
        # projections those normalizes unblock (chunks -2 and -1).
        for av in av_q:
            attnv(*av)
        for rfn in recq:
            rfn()
        recq = []
        for nfn in pend:
            nfn()
        pend = []
        for pq0, pqw in CHUNKS[-2:]:
            for nt in range(pq0 // P, (pq0 + pqw) // P):
                fillers.append(lambda nt=nt: fF_unit(nt))
        for f in fillers:
            f()


def build_program(n_cores: int = 8):
    nc = bacc.Bacc(
        "TRN2",
        target_bir_lowering=False,
        debug=False,
        enable_asserts=False,
        num_devices=n_cores,
    )
    d = {
        "xT": nc.dram_tensor("xT", [C, N], XT_DT, kind="ExternalInput").ap(),
        "wqP": nc.dram_tensor("wqP", [C, PAIRS * P], XT_DT, kind="ExternalInput").ap(),
        "wkP": nc.dram_tensor("wkP", [C, PAIRS * P], XT_DT, kind="ExternalInput").ap(),
        "wvA": nc.dram_tensor("wvA", [C, VW], XT_DT, kind="ExternalInput").ap(),
        "vbB": nc.dram_tensor("vbB", [P, VW], F32, kind="ExternalInput").ap(),
        "qbP": nc.dram_tensor("qbP", [P, PAIRS], F32, kind="ExternalInput").ap(),
        "kbP": nc.dram_tensor("kbP", [P, PAIRS], F32, kind="ExternalInput").ap(),
        "pwP": nc.dram_tensor("pwP", [PAIRS, P, C], OT_DT, kind="ExternalInput").ap(),
        "pbR": nc.dram_tensor("pbR", [P, C], F32, kind="ExternalInput").ap(),
        "selE2": nc.dram_tensor("selE2", [1, 256], BC_DT, kind="ExternalInput").ap(),
        "out": nc.dram_tensor("out", [N, C], F32, kind="ExternalOutput").ap(),
    }
    import contextlib

    with tile.TileContext(nc) as tc:
        with contextlib.ExitStack() as ctx:
            _emit(tc, d, ctx)
    nc.finalize()
    return nc


def _np_dtype(dt):
    if dt == mybir.dt.bfloat16:
        import ml_dtypes

        return ml_dtypes.bfloat16
    return np.float32


def _prep_host(x, q_w, q_b, kv_w, kv_b, proj_w, proj_b):
    """Transpose/pack on host. Returns (per-core xT list, shared map)."""
    f32 = np.float32
    x = np.asarray(x, f32)
    xT = np.ascontiguousarray(x.reshape(B, N, C).transpose(0, 2, 1))  # [B, C, N]

    qwT = np.ascontiguousarray(np.asarray(q_w, f32).T)  # [Cin, Cout]
    kwT = np.ascontiguousarray(np.asarray(kv_w[:C], f32).T)
    vwT = np.ascontiguousarray(np.asarray(kv_w[C:], f32).T)
    pwT = np.ascontiguousarray(np.asarray(proj_w, f32).T)

    wqP = np.zeros((C, PAIRS * P), f32)
    wkP = np.zeros((C, PAIRS * P), f32)
    qbP = np.zeros((P, PAIRS), f32)
    kbP = np.zeros((P, PAIRS), f32)
    pwP = np.zeros((PAIRS, P, C), f32)
    for p in range(PAIRS):
        a, b = 2 * p, 2 * p + 1
        wqP[:, p * P : p * P + HD] = qwT[:, a * HD : (a + 1) * HD]
        wqP[:, p * P + 64 : p * P + 64 + HD] = qwT[:, b * HD : (b + 1) * HD]
        wkP[:, p * P : p * P + HD] = kwT[:, a * HD : (a + 1) * HD]
        wkP[:, p * P + 64 : p * P + 64 + HD] = kwT[:, b * HD : (b + 1) * HD]
        qbP[0:HD, p] = q_b[a * HD : (a + 1) * HD]
        qbP[64 : 64 + HD, p] = q_b[b * HD : (b + 1) * HD]
        kbP[0:HD, p] = kv_b[a * HD : (a + 1) * HD]
        kbP[64 : 64 + HD, p] = kv_b[b * HD : (b + 1) * HD]
        # rows 1..48 / 65..112 carry the proj weights; rows 0 / 64 stay zero
        # to swallow the denominator row of outT.
        pwP[p, 1 : 1 + HD, :] = pwT[a * HD : (a + 1) * HD, :]
        pwP[p, 65 : 65 + HD, :] = pwT[b * HD : (b + 1) * HD, :]

    # V blocks are [ones | v0..v47] per head: the ones column comes from the
    # replicated bias tile (DVE add), weight column stays zero.
    wvA = np.zeros((C, VW), f32)
    vb = np.zeros((VW,), f32)
    for h in range(NH):
        wvA[:, h * (HD + 1) + 1 : (h + 1) * (HD + 1)] = vwT[:, h * HD : (h + 1) * HD]
        vb[h * (HD + 1) + 1 : (h + 1) * (HD + 1)] = kv_b[C + h * HD : C + (h + 1) * HD]
        vb[h * (HD + 1)] = 1.0
    vbB = np.tile(vb[None, :], (P, 1))

    selE2 = np.zeros((1, 256), f32)
    selE2[0, 0 : HD + 1] = 1.0
    selE2[0, 128 + 64 : 128 + 64 + HD + 1] = 1.0

    pbB = np.tile(np.asarray(proj_b, f32)[None, :], (P, 1))

    xdt = _np_dtype(XT_DT)
    odt = _np_dtype(OT_DT)
    shared = {
        "selE2": selE2,
        "wqP": wqP.astype(xdt),
        "wkP": wkP.astype(xdt),
        "wvA": wvA.astype(xdt),
        "vbB": vbB,
        "qbP": qbP,
        "kbP": kbP,
        "pwP": pwP.astype(odt),
        "pbR": pbB,
    }
    return xT.astype(xdt), shared


_PROGRAM = None


def _get_program():
    global _PROGRAM
    if _PROGRAM is None:
        _PROGRAM = build_program(B)
    return _PROGRAM


def kernel(x, q_w, q_b, kv_w, kv_b, proj_w, proj_b):
    xT, shared = _prep_host(x, q_w, q_b, kv_w, kv_b, proj_w, proj_b)
    nc = _get_program()
    in_maps = [dict(shared, xT=np.ascontiguousarray(xT[b])) for b in range(B)]
    res = run_bass_kernel_spmd(nc, in_maps, list(range(B)))
    outs = [np.asarray(res.results[i]["out"], np.float32) for i in range(B)]
    return np.stack(outs).reshape(B, HH, WW, C)


# revision 59
# speedup vs baseline: 1.2201x; 1.2201x over previous
"""Trainium2 Bass kernel for nn_Attention_17532056502607.

Multi-head self-attention (B=8, N=48*48=2304 tokens, C=384, 8 heads of 48):
    q = x @ q_w.T + q_b ; k,v = x @ kv_w.T + kv_b
    out = softmax(q k^T / sqrt(48)) v ; y = out @ proj_w.T + proj_b

Sharding: data-parallel, one batch element per NeuronCore (8 cores).

The kernel is ACT(exp)-throughput bound: 8 heads x 2304^2 scores = 42.5M
exps at 1 elem/cyc/lane @1.2GHz is a ~280us floor on the Scalar engine.
The design keeps one continuous exp stream on ACT and hides ALL other
work (projections, attn@V, output proj, normalize) under it:

  - S^T layout: scores [keys, q] via kT-tile.T @ qT, head pairs packed at
    partition rows 0-47 / 64-111 so row-group-disjoint matmuls stream
    concurrently in the PE array.
  - exp groups of 3x512 (or 6x256) PSUM banks, double-buffered (2 tags),
    so S^T(g+2) emission only waits exp(g): ACT never starves.
  - attn@V accumulates into a SINGLE psum bank: head a at rows 0-48,
    head b at rows 64-112 (col-group-disjoint -> concurrent), with a
    ones-column per head producing the softmax denominator at rows 0/64.
  - the attn@V queue is delayed 3 groups and spans chunk/pair boundaries
    so the exp stream never flushes.
  - x->q/k/v projections (bf16, FWL) and the output projection run as
    "filler" PE work popped one unit per exp group; v and pair-0 q/k run
    up front, pair p+1's q/k during pair p, proj during pair 3.
  - normalize: 1 DVE copy of rows 0:113, reciprocal_approx_fast of the
    two denominator rows, one rank-2 selector matmul broadcasts both
    reciprocals, one DVE multiply into bf16 oT_sb.
"""

import os
import sys

import numpy as np

for _p in ("/opt/trn_rl_repo",):
    if _p not in sys.path:
        sys.path.append(_p)

import concourse.bass as bass  # noqa: E402
import concourse.tile as tile  # noqa: E402
from concourse import bacc, mybir  # noqa: E402
from concourse.bass_utils import run_bass_kernel_spmd  # noqa: E402

# ---------------------------------------------------------------- constants
B = 8
HH = 48
WW = 48
C = 384
N = HH * WW  # 2304
NH = 8
HD = 48
PAIRS = NH // 2  # 4
P = 128
NT = N // P  # 18 token tiles
KTC = C // P  # 3 contraction tiles over C
SCALE = float(HD) ** -0.5
VW = NH * (HD + 1)  # 392: v with a ones column per head
CHUNKS = [(0, 512), (512, 512), (1024, 512), (1536, 512), (2048, 256)]

F32 = mybir.dt.float32
BF16 = mybir.dt.bfloat16
F32R = mybir.dt.float32r

# Input-side matmul dtype (x, q/k/v weights): bf16 halves DMA/SBUF and
# enables FWL weight loads.
XT_DT = getattr(mybir.dt, os.environ.get("ATTN_XT_DT", "bfloat16"))
# S^T operands (qT/kT) and attn@V operands (v, est).
ST_DT = getattr(mybir.dt, os.environ.get("ATTN_ST_DT", "bfloat16"))
AV_DT = getattr(mybir.dt, os.environ.get("ATTN_AV_DT", "bfloat16"))
# Normalized attention output + proj weights.
OT_DT = getattr(mybir.dt, os.environ.get("ATTN_OT_DT", "bfloat16"))
# Broadcast-matmul operands (rank-1 selectors): f32r streams at 1 cyc/col
# on the PE (fp32 is 4). The reciprocal writes f32r directly under
# allow_low_precision so the verifier sees a rounded producer.
BC_DT = F32R

_EXP = mybir.ActivationFunctionType.Exp


def _groups_for(parity):
    """List of (gs, tag) exp groups covering the 36 S^T tiles of a chunk.
    Tag 'A' tiles span 3 PSUM banks, tag 'B' tiles span 2 (5 banks total,
    strictly alternating — including across chunk boundaries via the
    parity flip — so consecutive groups never wait on each other's exp).
    Tiles are always 512 wide — at qw=256 each matmul still gets a full
    bank to itself (two concurrent row-group matmuls must not share a
    PSUM bank)."""
    if parity == 0:
        return [(3, "A"), (2, "B")] * 7 + [(1, "A")]
    return [(2, "B"), (3, "A")] * 7 + [(1, "B")]


def _emit(tc: tile.TileContext, d: dict, ctx):
    nc = tc.nc

    persist = ctx.enter_context(tc.tile_pool(name="persist", bufs=1))
    v_sb = persist.tile([P, NT, VW], AV_DT, name="v_sb")
    qT_sb = persist.tile([P, PAIRS, N], ST_DT, name="qT_sb")
    kT_sb = persist.tile([P, PAIRS, N], ST_DT, name="kT_sb")
    oT_sb = persist.tile([P, PAIRS, N], OT_DT, name="oT_sb")
    pw_sb = persist.tile([P, PAIRS, C], OT_DT, name="pw_sb")
    qb_sb = persist.tile([P, PAIRS], F32, name="qb_sb")
    kb_sb = persist.tile([P, PAIRS], F32, name="kb_sb")
    vbB_sb = persist.tile([P, VW], F32, name="vbB_sb")
    pbB_sb = persist.tile([P, C], F32, name="pbB_sb")
    sel2_sb = persist.tile([1, 256], BC_DT, name="sel2_sb")
    xT_sb = persist.tile([P, KTC, N], XT_DT, name="xT_sb")
    wq_sb = persist.tile([P, KTC, PAIRS * P], XT_DT, name="wq_sb")
    wk_sb = persist.tile([P, KTC, PAIRS * P], XT_DT, name="wk_sb")
    wv_sb = persist.tile([P, KTC, VW], XT_DT, name="wv_sb")

    # DMAs ordered by first use: v batch 0 needs xT slice kt and wv.
    for kt in range(KTC):
        nc.sync.dma_start(
            xT_sb[:, kt, :], d["xT"][kt * P : (kt + 1) * P, :]
        )
    nc.sync.dma_start(wv_sb[:], d["wvA"].rearrange("(kt p) m -> p kt m", p=P))
    nc.sync.dma_start(vbB_sb[:], d["vbB"])
    nc.sync.dma_start(wq_sb[:], d["wqP"].rearrange("(kt p) m -> p kt m", p=P))
    nc.sync.dma_start(wk_sb[:], d["wkP"].rearrange("(kt p) m -> p kt m", p=P))
    nc.sync.dma_start(qb_sb[:], d["qbP"])
    nc.sync.dma_start(kb_sb[:], d["kbP"])
    # (sel2/pw/pbB DMAs are emitted after phase A: they are needed much
    # later, and phase-A matmuls wait on the shared DMA semaphore.)

    # oT_sb rows 49-63 are never written by the normalize muls: zero the
    # whole tensor once, at t=0 while the DVE is idle (the proj matmul
    # contracts 0-weights against these rows). Also fire a tiny dummy exp
    # so the ~2.7us ACT table load happens during phase A, not at the
    # first real softmax.
    nc.vector.memset(oT_sb[:], 0.0)
    warm = persist.tile([1, 8], F32, name="warm")
    nc.vector.memset(warm[:], 0.0)
    nc.scalar.activation(warm[:], warm[:], _EXP, scale=1.0)

    # ------------- helpers emitting one "unit" of projection work ---------
    def qk_unit(pool, tag, wt, bias, dst, pr, q0, qw):
        ps = pool.tile([P, 512], F32, name="psqk", tag=tag)
        for kt in range(KTC):
            nc.tensor.matmul(
                ps[:, 0:qw],
                lhsT=wt[:, kt, pr * P : (pr + 1) * P],
                rhs=xT_sb[:, kt, q0 : q0 + qw],
                start=(kt == 0),
                stop=(kt == KTC - 1),
            )
        nc.vector.tensor_scalar_add(
            dst[:, pr, q0 : q0 + qw], ps[:, 0:qw], bias[:, pr : pr + 1]
        )

    def v_unit(pool, tag, nt):
        psv = pool.tile([P, 512], F32, name="psv", tag=tag)
        for kt in range(KTC):
            nc.tensor.matmul(
                psv[:, 0:VW],
                lhsT=xT_sb[:, kt, nt * P : (nt + 1) * P],
                rhs=wv_sb[:, kt, :],
                start=(kt == 0),
                stop=(kt == KTC - 1),
            )
        nc.vector.tensor_add(v_sb[:, nt, :], psv[:, 0:VW], vbB_sb[:])

    # ---------------- phase A upfront: v batch 0, kT pair 0, qT c0/c1 -----
    DBG_NOSTREAM = os.environ.get("ATTN_DBG_NOSTREAM") == "1"
    with tc.tile_pool(name="phA", bufs=1, space="PSUM") as psA:
        # v token tiles 0-5, kt-outer across 6 open accumulators so the
        # first matmuls start as soon as xT slice 0 lands.
        nbat = 3 if DBG_NOSTREAM else 1
        for b in range(nbat):
            psv = [
                psA.tile([P, 512], F32, name=f"psv{i}", tag=f"b{i}")
                for i in range(6)
            ]
            for kt in range(KTC):
                for i in range(6):
                    nc.tensor.matmul(
                        psv[i][:, 0:VW],
                        lhsT=xT_sb[:, kt, (b * 6 + i) * P : (b * 6 + i + 1) * P],
                        rhs=wv_sb[:, kt, :],
                        start=(kt == 0),
                        stop=(kt == KTC - 1),
                    )
            for i in range(6):
                nc.vector.tensor_add(
                    v_sb[:, b * 6 + i, :], psv[i][:, 0:VW], vbB_sb[:]
                )
        # kT pair 0 (all key chunks: chunk 0's S^T sweeps every key tile),
        # qT pair 0 chunks 0-1.
        for q0, qw in CHUNKS:
            qk_unit(psA, "u0", wk_sb, kb_sb, kT_sb, 0, q0, qw)
        for q0, qw in CHUNKS[:2]:
            qk_unit(psA, "u1", wq_sb, qb_sb, qT_sb, 0, q0, qw)
        if DBG_NOSTREAM:
            for q0, qw in CHUNKS[2:]:
                qk_unit(psA, "u1", wq_sb, qb_sb, qT_sb, 0, q0, qw)
            for prn in range(1, PAIRS):
                for q0, qw in CHUNKS:
                    qk_unit(psA, "u0", wk_sb, kb_sb, kT_sb, prn, q0, qw)
                    qk_unit(psA, "u1", wq_sb, qb_sb, qT_sb, prn, q0, qw)

    # late DMAs: first consumed by the normalize (~40us) and proj (~100us)
    nc.sync.dma_start(sel2_sb[:], d["selE2"])
    nc.sync.dma_start(pw_sb[:], d["pwP"].rearrange("r p m -> p r m"))
    nc.sync.dma_start(pbB_sb[:], d["pbR"])

    # ---------------- fused attention + filler stream ---------------------
    with (
        tc.tile_pool(name="es", bufs=5) as es_pool,
        tc.tile_pool(name="rc", bufs=4) as rc_pool,
        tc.tile_pool(name="fin", bufs=3) as fin_pool,
        tc.tile_pool(name="psS", bufs=1, space="PSUM") as psS,
        tc.tile_pool(name="psO", bufs=1, space="PSUM") as psO,
        tc.tile_pool(name="psX", bufs=1, space="PSUM") as psX,
    ):
        # attn@V accumulator banks: head a at rows 0-48 of oTA, head b at
        # rows 64-112 of oTB (disjoint col groups -> concurrent matmuls;
        # separate banks because a PSUM bank allows one open accumulation
        # group at a time). oT_sb rows 49-63 are never written: zero once
        # so the proj matmul contracts 0-weights against 0, not garbage.
        oTA = psO.tile([P, 512], F32, name="oTA", tag="oTA")
        oTB = psO.tile([P, 512], F32, name="oTB", tag="oTB")
        # seed rows 49-63 of both oU slots with 1.0 once: the drain copies
        # never write them, and the [0:65] batched reciprocal must not see
        # zeros (1/0 faults the DVE) — the baseline seeded its den tiles
        # the same way.
        for _i in range(4):
            _oUi = rc_pool.tile([P, 512], F32, name=f"oUinit{_i}", tag="oU")
            nc.vector.memset(_oUi[32:64, :], 1.0)

        def fF_unit(nt):
            fF = psX.tile([P, C], F32, name="fF", tag="scr")
            for pr2 in range(PAIRS):
                nc.tensor.matmul(
                    fF[:],
                    lhsT=oT_sb[0:113, pr2, nt * P : (nt + 1) * P],
                    rhs=pw_sb[0:113, pr2, :],
                    start=(pr2 == 0),
                    stop=(pr2 == PAIRS - 1),
                )
            ft = fin_pool.tile([P, C], F32, name="ft", tag="ft")
            nc.vector.tensor_add(ft[:], fF[:], pbB_sb[:])
            nc.sync.dma_start(d["out"][nt * P : (nt + 1) * P, :], ft[:])

        # filler queue: each entry emits one small batch of PE work whose
        # inputs are already (or will shortly be) available. Consumers of
        # a filler's output must be EMITTED after it (in-order PE queue).
        fillers = []
        if not DBG_NOSTREAM:
            # v tiles 6-17 during pair-0 chunk 0 (needed by its attn@V sweep)
            for nt in range(6, NT):
                fillers.append(lambda nt=nt: v_unit(psX, "scr", nt))
            # qT pair-0 chunks 2-4 (needed at chunks 2-4).
            for q0, qw in CHUNKS[2:]:
                fillers.append(
                    lambda q0=q0, qw=qw: qk_unit(
                        psX, "scr", wq_sb, qb_sb, qT_sb, 0, q0, qw
                    )
                )
        # pair p+1's k/q during pair p, proj during pair 3: appended below.

        seq = [(kt, hoff) for kt in range(NT) for hoff in (0, 64)]
        av_q = []  # delayed attn@V emissions, spans chunk/pair boundaries
        pend = []  # queued normalize closures (emitted 2 chunks later)
        recq = []  # queued reciprocal closures (emitted 1 chunk later)

        def attnv(est, si, gs, pr, qw, after):
            for j in range(gs):
                kt2, hoff2 = seq[si + j]
                h = pr * 2 + (0 if hoff2 == 0 else 1)
                oT = oTA if hoff2 == 0 else oTB
                nc.tensor.matmul(
                    oT[hoff2 : hoff2 + HD + 1, 0:qw],
                    lhsT=v_sb[:, kt2, h * (HD + 1) : (h + 1) * (HD + 1)],
                    rhs=est[:, j, 0:qw],
                    start=(kt2 == 0),
                    stop=(kt2 == NT - 1),
                )
            if after is not None:
                after()

        def make_drain(pr, q0, qw):
            # emitted right after the chunk's last attn@V: DVE copies of the
            # two head blocks, then fast approximate reciprocals of the two
            # denominator rows. The normalize (PE broadcast + DVE muls) is
            # deferred via pend[].
            def drain():
                # only the two copies sit on the oTA/oTB WAR path (the
                # next chunk's attn@V waits on them); the reciprocal reads
                # the SBUF copy so the accumulator banks free immediately,
                # and is emitted at the END of the next chunk's group loop
                # so no filler add (whose completion gates the scr-bank
                # WAR for the bc matmul) ever queues behind its ~3.4us
                # DVE occupancy. (reciprocal_approx_fast is NOT used: the
                # custom-DVE op executes as garbage through this runtime.)
                oU = rc_pool.tile([P, 512], F32, name="oU", tag="oU")
                nc.vector.tensor_copy(oU[0 : HD + 1, 0:qw], oTA[0 : HD + 1, 0:qw])
                nc.vector.tensor_copy(
                    oU[64 : 64 + HD + 1, 0:qw], oTB[64 : 64 + HD + 1, 0:qw]
                )
                rec = rc_pool.tile([P, 512], BC_DT, name="rec", tag="rec")
                rec1 = rc_pool.tile([1, 512], BC_DT, name="rec1", tag="rec1")
                # one reciprocal spans both denominator rows (0, 64);
                # rows 1-63 are junk reciprocals, never read. ([1,512]
                # reciprocals measure ~2x slower than this shape.)
                with nc.allow_low_precision(
                    reason="float32r keeps fp32 bits; PE rounds on read"
                ):
                    nc.vector.reciprocal(rec[0:65, 0:qw], oU[0:65, 0:qw])
                # recB moved to a base-0 row: both selector matmuls then
                # keep lhsT/rhs/dst at partition base 0 (f32r cannot
                # write PSUM at base 64).
                nc.vector.tensor_copy(rec1[0:1, 0:qw], rec[64:65, 0:qw])

                def normalize():
                    if os.environ.get("ATTN_DBG_NONORM") == "1":
                        nc.vector.tensor_copy(
                            oT_sb[0 : HD + 1, pr, q0 : q0 + qw], oU[0 : HD + 1, 0:qw]
                        )
                        nc.vector.tensor_copy(
                            oT_sb[64 : 64 + HD + 1, pr, q0 : q0 + qw],
                            oU[64 : 64 + HD + 1, 0:qw],
                        )
                        return
                    # rank-1 selector matmuls broadcast recA to bc rows
                    # 0-48 and recB to rows 64-112, accumulating into a
                    # base-0 dst (f32r cannot write PSUM at base 64).
                    bc = psX.tile([P, 512], F32, name="bc", tag="scr")
                    nc.tensor.matmul(
                        bc[0:113, 0:qw],
                        lhsT=sel2_sb[0:1, 0:113],
                        rhs=rec[0:1, 0:qw],
                        start=True,
                        stop=False,
                    )
                    nc.tensor.matmul(
                        bc[0:113, 0:qw],
                        lhsT=sel2_sb[0:1, 128:241],
                        rhs=rec1[0:1, 0:qw],
                        start=False,
                        stop=True,
                    )
                    nc.vector.tensor_mul(
                        oT_sb[0 : HD + 1, pr, q0 : q0 + qw],
                        oU[0 : HD + 1, 0:qw],
                        bc[0 : HD + 1, 0:qw],
                    )
                    nc.vector.tensor_mul(
                        oT_sb[64 : 64 + HD + 1, pr, q0 : q0 + qw],
                        oU[64 : 64 + HD + 1, 0:qw],
                        bc[64 : 64 + HD + 1, 0:qw],
                    )

                pend.append(normalize)

            return drain

        _maxit = int(os.environ.get("ATTN_DBG_MAXIT", "20"))
        _chunks = list(CHUNKS)
        if os.environ.get("ATTN_DBG_DUP512") == "1":
            _chunks[-1] = _chunks[-2]
        _it = 0
        for pr in range(PAIRS):
            for ci, (q0, qw) in enumerate(_chunks):
                if _it >= _maxit:
                    break
                _it += 1
                groups = _groups_for((pr * len(CHUNKS) + ci) % 2)
                pend_idx = len(groups) - 3
                si = 0
                for g, (gs, btag) in enumerate(groups):
                    if g == pend_idx and len(pend) >= 2:
                        pend.pop(0)()
                        if pr == PAIRS - 1 and ci >= 2:
                            # proj for the chunk just normalized
                            pq0, pqw = CHUNKS[ci - 2]
                            for nt in range(pq0 // P, (pq0 + pqw) // P):
                                fillers.append(lambda nt=nt: fF_unit(nt))
                    sg = psS.tile(
                        [P, gs, 512], F32, name="sg", tag=f"sg{btag}"
                    )
                    for j in range(gs):
                        kt, hoff = seq[si + j]
                        nc.tensor.matmul(
                            sg[:, j, 0:qw],
                            lhsT=kT_sb[hoff : hoff + HD, pr, kt * P : (kt + 1) * P],
                            rhs=qT_sb[hoff : hoff + HD, pr, q0 : q0 + qw],
                            start=True,
                            stop=True,
                        )
                    est = es_pool.tile([P, gs, 512], AV_DT, name="est", tag="est")
                    nc.scalar.activation(
                        est[:, :, 0:qw], sg[:, :, 0:qw], _EXP, scale=SCALE
                    )
                    after = (
                        make_drain(pr, q0, qw) if g == len(groups) - 1 else None
                    )
                    av_q.append((est, si, gs, pr, qw, after))
                    si += gs
                    if len(av_q) > 3:
                        attnv(*av_q.pop(0))
                    # keep boundary slots clean: the PE must race ahead on
                    # the next chunk's first S^T groups and the queued
                    # attn@V tail there.
                    if g >= 2 and fillers:
                        fillers.pop(0)()
                if DBG_NOSTREAM:
                    # flush the attn@V queue at every chunk boundary
                    for av in av_q:
                        attnv(*av)
                    av_q = []
                # queue pair pr+1's projections during pair pr's chunks
                # (2 units per chunk: all 10 done before pair pr+1 starts)
                if not DBG_NOSTREAM and pr < PAIRS - 1:
                    q0n, qwn = CHUNKS[ci]
                    fillers.append(
                        lambda q0n=q0n, qwn=qwn, prn=pr + 1: qk_unit(
                            psX, "scr", wk_sb, kb_sb, kT_sb, prn, q0n, qwn
                        )
                    )
                    fillers.append(
                        lambda q0n=q0n, qwn=qwn, prn=pr + 1: qk_unit(
                            psX, "scr", wq_sb, qb_sb, qT_sb, prn, q0n, qwn
                        )
                    )

        # tail: flush the attn@V queue, the two queued normalizes, and the
        # projections those normalizes unblock (chunks -2 and -1).
        for av in av_q:
            attnv(*av)
        for rfn in recq:
            rfn()
        recq = []
        for nfn in pend:
            nfn()
        pend = []
        for pq0, pqw in CHUNKS[-2:]:
            for nt in range(pq0 // P, (pq0 + pqw) // P):
                fillers.append(lambda nt=nt: fF_unit(nt))
        for f in fillers:
            f()


def build_program(n_cores: int = 8):
    nc = bacc.Bacc(
        "TRN2",
        target_bir_lowering=False,
        debug=False,
        enable_asserts=False,
        num_devices=n_cores,
    )
    d = {
        "xT": nc.dram_tensor("xT", [C, N], XT_DT, kind="ExternalInput").ap(),
        "wqP": nc.dram_tensor("wqP", [C, PAIRS * P], XT_DT, kind="ExternalInput").ap(),
        "wkP": nc.dram_tensor("wkP", [C, PAIRS * P], XT_DT, kind="ExternalInput").ap(),
        "wvA": nc.dram_tensor("wvA", [C, VW], XT_DT, kind="ExternalInput").ap(),
        "vbB": nc.dram_tensor("vbB", [P, VW], F32, kind="ExternalInput").ap(),
        "qbP": nc.dram_tensor("qbP", [P, PAIRS], F32, kind="ExternalInput").ap(),
        "kbP": nc.dram_tensor("kbP", [P, PAIRS], F32, kind="ExternalInput").ap(),
        "pwP": nc.dram_tensor("pwP", [PAIRS, P, C], OT_DT, kind="ExternalInput").ap(),
        "pbR": nc.dram_tensor("pbR", [P, C], F32, kind="ExternalInput").ap(),
        "selE2": nc.dram_tensor("selE2", [1, 256], BC_DT, kind="ExternalInput").ap(),
        "out": nc.dram_tensor("out", [N, C], F32, kind="ExternalOutput").ap(),
    }
    import contextlib

    with tile.TileContext(nc) as tc:
        with contextlib.ExitStack() as ctx:
            _emit(tc, d, ctx)
    nc.finalize()
    return nc


def _np_dtype(dt):
    if dt == mybir.dt.bfloat16:
        import ml_dtypes

        return ml_dtypes.bfloat16
    return np.float32


def _prep_host(x, q_w, q_b, kv_w, kv_b, proj_w, proj_b):
    """Transpose/pack on host. Returns (per-core xT list, shared map)."""
    f32 = np.float32
    x = np.asarray(x, f32)
    xT = np.ascontiguousarray(x.reshape(B, N, C).transpose(0, 2, 1))  # [B, C, N]

    qwT = np.ascontiguousarray(np.asarray(q_w, f32).T)  # [Cin, Cout]
    kwT = np.ascontiguousarray(np.asarray(kv_w[:C], f32).T)
    vwT = np.ascontiguousarray(np.asarray(kv_w[C:], f32).T)
    pwT = np.ascontiguousarray(np.asarray(proj_w, f32).T)

    wqP = np.zeros((C, PAIRS * P), f32)
    wkP = np.zeros((C, PAIRS * P), f32)
    qbP = np.zeros((P, PAIRS), f32)
    kbP = np.zeros((P, PAIRS), f32)
    pwP = np.zeros((PAIRS, P, C), f32)
    for p in range(PAIRS):
        a, b = 2 * p, 2 * p + 1
        wqP[:, p * P : p * P + HD] = qwT[:, a * HD : (a + 1) * HD]
        wqP[:, p * P + 64 : p * P + 64 + HD] = qwT[:, b * HD : (b + 1) * HD]
        wkP[:, p * P : p * P + HD] = kwT[:, a * HD : (a + 1) * HD]
        wkP[:, p * P + 64 : p * P + 64 + HD] = kwT[:, b * HD : (b + 1) * HD]
        qbP[0:HD, p] = q_b[a * HD : (a + 1) * HD]
        qbP[64 : 64 + HD, p] = q_b[b * HD : (b + 1) * HD]
        kbP[0:HD, p] = kv_b[a * HD : (a + 1) * HD]
        kbP[64 : 64 + HD, p] = kv_b[b * HD : (b + 1) * HD]
        # rows 1..48 / 65..112 carry the proj weights; rows 0 / 64 stay zero
        # to swallow the denominator row of outT.
        pwP[p, 1 : 1 + HD, :] = pwT[a * HD : (a + 1) * HD, :]
        pwP[p, 65 : 65 + HD, :] = pwT[b * HD : (b + 1) * HD, :]

    # V blocks are [ones | v0..v47] per head: the ones column comes from the
    # replicated bias tile (DVE add), weight column stays zero.
    wvA = np.zeros((C, VW), f32)
    vb = np.zeros((VW,), f32)
    for h in range(NH):
        wvA[:, h * (HD + 1) + 1 : (h + 1) * (HD + 1)] = vwT[:, h * HD : (h + 1) * HD]
        vb[h * (HD + 1) + 1 : (h + 1) * (HD + 1)] = kv_b[C + h * HD : C + (h + 1) * HD]
        vb[h * (HD + 1)] = 1.0
    vbB = np.tile(vb[None, :], (P, 1))

    selE2 = np.zeros((1, 256), f32)
    selE2[0, 0 : HD + 1] = 1.0
    selE2[0, 128 + 64 : 128 + 64 + HD + 1] = 1.0

    pbB = np.tile(np.asarray(proj_b, f32)[None, :], (P, 1))

    xdt = _np_dtype(XT_DT)
    odt = _np_dtype(OT_DT)
    shared = {
        "selE2": selE2,
        "wqP": wqP.astype(xdt),
        "wkP": wkP.astype(xdt),
        "wvA": wvA.astype(xdt),
        "vbB": vbB,
        "qbP": qbP,
        "kbP": kbP,
        "pwP": pwP.astype(odt),
        "pbR": pbB,
    }
    return xT.astype(xdt), shared


_PROGRAM = None


def _get_program():
    global _PROGRAM
    if _PROGRAM is None:
        _PROGRAM = build_program(B)
    return _PROGRAM


def kernel(x, q_w, q_b, kv_w, kv_b, proj_w, proj_b):
    xT, shared = _prep_host(x, q_w, q_b, kv_w, kv_b, proj_w, proj_b)
    nc = _get_program()
    in_maps = [dict(shared, xT=np.ascontiguousarray(xT[b])) for b in range(B)]
    res = run_bass_kernel_spmd(nc, in_maps, list(range(B)))
    outs = [np.asarray(res.results[i]["out"], np.float32) for i in range(B)]
    return np.stack(outs).reshape(B, HH, WW, C)
